# revision 12
# baseline (speedup 1.0000x reference)
"""Trainium2 Bass kernel for a 4-layer LIF spiking net (snnTorch-style Leaky).

Reference (per step t, B=256, T=100):
    cur0 = x_t @ W0.T + b0 ; lif0 (1024)
    cur1 = s0 @ W1.T + b1  ; lif1 (512)
    cur2 = s1 @ W2.T + b2  ; lif2 (256)
    cur3 = s2 @ W3.T + b3  ; lif3 (10)   -> outputs (s3, m3) per step
    lif:  m' = 0.95*m + cur - H(m_prev - 1);  s = H(m' - 1)

Strategy: data-parallel over batch (32/core x 8 cores).
 - Numerics: all matmuls run on the PE in fp16 *pair* form, which is
   accurate to ~fp32 rounding class: operands are split hi/lo
   (a = a_h + a_l with a_h = fp16(a)), weights as W = W_h + W_l*2^-11
   with W_l stored prescaled by 2^11 (keeps it fp16-normal).  Spikes are
   {0,1} (exact in fp16), so layers 1-3 need only 2 passes; layer 0
   needs 3 (x_h*W_h, x_l*W_h, (x_h*2^-11)*(W_l*2^11)).
 - Phase 1 (time-parallel): CUR0 = W0 @ x.T + b0 for all t, produced in
   16-step chunks into an SBUF ring, interleaved with phase 2 steps.
 - Phase 2 (sequential over t): feature-major activations [D, 32].
   Matmuls use spike tiles as the stationary operand (M=32) streaming
   W.T (N=512/256); batch-major outputs are transposed back to
   feature-major on the PE.  LIF state uses the bias-shift u = m - 20*b
   (per-feature thresholds 1-20*b), so no per-step bias work.
 - Layer 3 keeps state u3 = m3 - 20*b3 in the output accumulator; one
   final elementwise add (+20*b3) converts the whole mem history before
   DMA-out.
"""
import sys
import numpy as np
from contextlib import ExitStack

sys.path.insert(0, '/opt/trn_rl_repo')

from concourse import bass, bacc, mybir, tile  # noqa: E402
from concourse.bass_utils import run_bass_kernel_spmd  # noqa: E402

F32 = mybir.dt.float32
F16 = mybir.dt.float16
ALU = mybir.AluOpType
ACTF = mybir.ActivationFunctionType

N_CORES = 8
BC = 32                      # batch per core
T = 100
D0, D1, D2, D3, D4 = 1024, 1024, 512, 256, 10
BETA = 0.95
SC = float(2.0 ** -11)       # lo-pass spike scale
CH = 16                      # time steps per phase-1 chunk
N_CH = (T + CH - 1) // CH    # 7 chunks (6x16 + 1x4)

M0 = D1 // 128   # 8  phase-1 M tiles
K0 = D0 // 128   # 8  phase-1 K tiles
F1 = D2 // 128   # 4  layer-1 feature folds
F2 = D3 // 128   # 2  layer-2 feature folds

_CACHE = {}


def _build():
    nc = bacc.Bacc("TRN2", target_bir_lowering=False)

    def dram(name, shape, dt, out=False):
        return nc.declare_dram_parameter(name, shape, dt, isOutput=out)

    x1d = dram("x1", [D0, T * BC], F16)
    x2d = dram("x2", [D0, T * BC], F16)
    x1sd = dram("x1s", [D0, T * BC], F16)
    w0hd = dram("w0h", [D0, D1], F16)
    w0ld = dram("w0l", [D0, D1], F16)
    w1hd = dram("w1h", [D1, D2], F16)
    w1ld = dram("w1l", [D1, D2], F16)
    w2hd = dram("w2h", [D2, D3], F16)
    w2ld = dram("w2l", [D2, D3], F16)
    w3hd = dram("w3h", [D3, D4], F16)
    w3ld = dram("w3l", [D3, D4], F16)
    b0rd = dram("b0r", [128, M0], F32)
    th1d = dram("th1f", [128, F1 * BC], F32)
    a1d = dram("a1f", [128, F1 * BC], F32)
    th2d = dram("th2f", [128, F2 * BC], F32)
    a2d = dram("a2f", [128, F2 * BC], F32)
    th3d = dram("th3r", [BC, D4], F32)
    a3d = dram("a3i", [BC, D4], F32)
    k3d = dram("k3r", [BC, T * D4], F32)
    eyed = dram("eye", [BC, BC], F32)
    ospk = dram("ospk", [BC, T * D4], F32, out=True)
    omem = dram("omem", [BC, T * D4], F32, out=True)

    with tile.TileContext(nc) as tc, ExitStack() as ctx:
        const = ctx.enter_context(tc.tile_pool(name="const", bufs=1))
        state = ctx.enter_context(tc.tile_pool(name="state", bufs=1))
        ring = ctx.enter_context(tc.tile_pool(name="ring", bufs=3))
        xin = ctx.enter_context(tc.tile_pool(name="xin", bufs=40))
        sbt = ctx.enter_context(tc.tile_pool(name="sbt", bufs=5))
        ps = ctx.enter_context(tc.tile_pool(name="ps", bufs=1, space="PSUM"))

        # ---------------- constants ----------------
        def loadrows(name, src, n_k, width, dt):
            ts_ = []
            for k in range(n_k):
                t_ = const.tile([128, width], dt, tag=f"{name}{k}", name=f"{name}{k}")
                nc.sync.dma_start(t_[:], src[k * 128:(k + 1) * 128, :])
                ts_.append(t_)
            return ts_

        w0h = loadrows("w0h", w0hd, K0, D1, F16)

        def load1(name, src, shape, dt):
            t_ = const.tile(shape, dt, tag=name, name=name)
            nc.sync.dma_start(t_[:], src[:])
            return t_

        b0r_sb = load1("b0r", b0rd, [128, M0], F32)
        th1_sb = load1("th1", th1d, [128, F1 * BC], F32)
        th2_sb = load1("th2", th2d, [128, F2 * BC], F32)
        th3_sb = load1("th3", th3d, [BC, D4], F32)
        k3_sb = load1("k3", k3d, [BC, T * D4], F32)
        eye_sb = load1("eye", eyed, [BC, BC], F32)

        # ---------------- state ----------------
        def ppair(name, shape, dt=F32):
            return [state.tile(shape, dt, tag=f"{name}{i}", name=f"{name}{i}")
                    for i in range(2)]

        u0 = ppair("u0", [128, M0 * BC])
        A0 = ppair("A0", [128, M0 * BC])
        s0h = ppair("s0h", [128, M0 * BC], F16)
        s0s = ppair("s0s", [128, M0 * BC], F16)
        u1 = ppair("u1", [128, F1 * BC])
        A1 = ppair("A1", [128, F1 * BC])
        s1h = ppair("s1h", [128, F1 * BC], F16)
        s1s = ppair("s1s", [128, F1 * BC], F16)
        u2 = ppair("u2", [128, F2 * BC])
        A2 = ppair("A2", [128, F2 * BC])
        s2h = ppair("s2h", [128, F2 * BC], F16)
        s2s = ppair("s2s", [128, F2 * BC], F16)
        spk_acc = state.tile([BC, (T + 1) * D4], F32, tag="spk_acc")
        mem_acc = state.tile([BC, (T + 1) * D4], F32, tag="mem_acc")

        nc.vector.memset(A0[1][:], 0.0)
        nc.sync.dma_start(A1[1][:], a1d[:])
        nc.sync.dma_start(A2[1][:], a2d[:])
        nc.vector.memset(spk_acc[:, 0:D4], 0.0)
        nc.sync.dma_start(mem_acc[:, 0:D4], a3d[:])

        ring_tiles = {}

        # deferred constants (not needed until step 0)
        w1h = loadrows("w1h", w1hd, M0, D2, F16)
        w1l = loadrows("w1l", w1ld, M0, D2, F16)
        w2h = loadrows("w2h", w2hd, F1, D3, F16)
        w2l = loadrows("w2l", w2ld, F1, D3, F16)
        w3h = loadrows("w3h", w3hd, F2, D4, F16)
        w3l = loadrows("w3l", w3ld, F2, D4, F16)

        # ---------------- phase-1 (emitted as per-m units) ----------------
        chunk_x = {}

        def chunk_begin(c):
            t0 = c * CH
            nT = min(CH, T - t0)
            n = nT * BC
            xt = []
            for src_, eng in ((x1d, nc.sync), (x2d, nc.sync),
                              (x1sd, nc.sync)):
                row = []
                for k in range(K0):
                    t_ = xin.tile([128, CH * BC], F16, tag="x", name="xt")
                    eng.dma_start(t_[:, 0:n],
                                  src_[k * 128:(k + 1) * 128,
                                       t0 * BC:t0 * BC + n])
                    row.append(t_)
                xt.append(row)
            chunk_x[c] = (xt, nT, n)
            cur = ring.tile([128, CH, M0, BC], F32, tag="cur0", name="cur")
            ring_tiles[c] = cur

        def chunk_m(c, m):
            xt, nT, n = chunk_x[c]
            cur = ring_tiles[c]
            pp = ps.tile([128, 512], F32, tag="ph1", bufs=2, name="pp")
            for k in range(K0):
                ms = slice(m * 128, (m + 1) * 128)
                nc.tensor.matmul(pp[:, 0:n], w0h[k][:, ms], xt[0][k][:, 0:n],
                                 start=(k == 0), stop=False)
                nc.tensor.matmul(pp[:, 0:n], w0h[k][:, ms], xt[1][k][:, 0:n],
                                 start=False, stop=False)
            for k in range(K0):
                ms = slice(m * 128, (m + 1) * 128)
                nc.tensor.matmul(pp[:, 0:n], w0l[k][:, ms], xt[2][k][:, 0:n],
                                 start=False, stop=(k == K0 - 1))
            nc.scalar.activation(
                cur[:, 0:nT, m, :],
                pp[:, 0:n].rearrange("p (a b) -> p a b", b=BC),
                ACTF.Identity, bias=b0r_sb[:, m:m + 1], scale=1.0)

        # ---------------- step stages ----------------
        pend1 = {}
        pend2 = {}
        pt1s = {}
        pt2s = {}
        pm3s = {}

        def a_dve(t):
            # layer-0 LIF (critical recurrence, runs 1 step ahead)
            cur = ring_tiles[t // CH]
            ti = t % CH
            w_, r_ = t % 2, (t + 1) % 2
            c0 = cur[:, ti, :, :].rearrange("p a b -> p (a b)")
            nc.vector.tensor_tensor(u0[w_][:], A0[r_][:], c0, ALU.add)
            nc.vector.tensor_scalar(s0h[w_][:], u0[w_][:], 1.0, None, ALU.is_gt)
            nc.scalar.activation(s0s[w_][:], s0h[w_][:], ACTF.Copy,
                                 bias=0.0, scale=SC)
            nc.vector.scalar_tensor_tensor(A0[w_][:], u0[w_][:], BETA, s0h[w_][:],
                                           ALU.mult, ALU.subtract)

        def a_mm(t):
            # layer-1 matmuls + PSUM->SBUF copy
            w_ = t % 2
            pc1 = ps.tile([32, 512], F32, tag="curX", bufs=2, name="pc1")
            for m in range(M0):
                bs = slice(m * BC, (m + 1) * BC)
                nc.tensor.matmul(pc1[:], s0h[w_][:, bs], w1h[m][:],
                                 start=(m == 0), stop=False)
                nc.tensor.matmul(pc1[:], s0s[w_][:, bs], w1l[m][:],
                                 start=False, stop=(m == M0 - 1))
            c1sb = sbt.tile([32, 512], F32, tag="c1sb", bufs=4, name="c1sb")
            nc.scalar.copy(c1sb[:], pc1[:])
            pend1[t] = c1sb

        def b1(t):
            # transpose cur1 to feature-major
            c1sb = pend1.pop(t)
            pt1 = ps.tile([128, 128], F32, tag="fm", bufs=4, name="pt1")
            for j in range(F1):
                nc.tensor.transpose(pt1[:, j * BC:(j + 1) * BC],
                                    c1sb[:, j * 128:(j + 1) * 128],
                                    eye_sb[:])
            pt1s[t] = pt1

        def b2(t):
            # layer-1 LIF + layer-2 matmuls + copy
            w_, r_ = t % 2, (t + 1) % 2
            pt1 = pt1s.pop(t)
            nc.vector.tensor_tensor(u1[w_][:], A1[r_][:], pt1[:], ALU.add)
            nc.vector.tensor_tensor(s1h[w_][:], u1[w_][:], th1_sb[:], ALU.is_gt)
            nc.scalar.activation(s1s[w_][:], s1h[w_][:], ACTF.Copy,
                                 bias=0.0, scale=SC)
            nc.vector.scalar_tensor_tensor(A1[w_][:], u1[w_][:], BETA, s1h[w_][:],
                                           ALU.mult, ALU.subtract)
            pc2 = ps.tile([32, 512], F32, tag="curX", bufs=2, name="pc2")
            for j in range(F1):
                bs = slice(j * BC, (j + 1) * BC)
                nc.tensor.matmul(pc2[:, 0:D3], s1h[w_][:, bs], w2h[j][:],
                                 start=(j == 0), stop=False)
                nc.tensor.matmul(pc2[:, 0:D3], s1s[w_][:, bs], w2l[j][:],
                                 start=False, stop=(j == F1 - 1))
            c2sb = sbt.tile([32, D3], F32, tag="c2sb", bufs=4, name="c2sb")
            nc.scalar.copy(c2sb[:], pc2[:, 0:D3])
            pend2[t] = c2sb

        def c1(t):
            # transpose cur2
            c2sb = pend2.pop(t)
            pt2 = ps.tile([128, 128], F32, tag="fm", bufs=4, name="pt2")
            for j in range(F2):
                nc.tensor.transpose(pt2[:, j * BC:(j + 1) * BC],
                                    c2sb[:, j * 128:(j + 1) * 128],
                                    eye_sb[:])
            pt2s[t] = pt2

        def c2(t):
            # layer-2 LIF
            w_, r_ = t % 2, (t + 1) % 2
            pt2 = pt2s.pop(t)
            nc.vector.tensor_tensor(u2[w_][:], A2[r_][:], pt2[:, 0:F2 * BC],
                                    ALU.add)
            nc.vector.tensor_tensor(s2h[w_][:], u2[w_][:], th2_sb[:], ALU.is_gt)
            nc.scalar.activation(s2s[w_][:], s2h[w_][:], ACTF.Copy,
                                 bias=0.0, scale=SC)
            nc.vector.scalar_tensor_tensor(A2[w_][:], u2[w_][:], BETA, s2h[w_][:],
                                           ALU.mult, ALU.subtract)

        def c3(t):
            # layer-3 matmuls + LIF (outputs)
            w_ = t % 2
            pm3 = ps.tile([128, 128], F32, tag="fm", bufs=4, name="pm3")
            for j in range(F2):
                bs = slice(j * BC, (j + 1) * BC)
                nc.tensor.matmul(pm3[0:BC, 0:D4], s2h[w_][:, bs], w3h[j][:],
                                 start=(j == 0), stop=False)
                nc.tensor.matmul(pm3[0:BC, 0:D4], s2s[w_][:, bs], w3l[j][:],
                                 start=False, stop=(j == F2 - 1))
            prev = slice(t * D4, (t + 1) * D4)
            cursl = slice((t + 1) * D4, (t + 2) * D4)
            pre3 = sbt.tile([BC, D4], F32, tag="pre3", bufs=3, name="pre3")
            nc.vector.scalar_tensor_tensor(pre3[:], mem_acc[:, prev], BETA,
                                           spk_acc[:, prev],
                                           ALU.mult, ALU.subtract)
            nc.vector.tensor_tensor(mem_acc[:, cursl], pre3[:],
                                    pm3[0:BC, 0:D4], ALU.add)
            nc.vector.tensor_tensor(spk_acc[:, cursl], mem_acc[:, cursl],
                                    th3_sb[:], ALU.is_gt)

        # ---------------- schedule (software-pipelined, staged leads) ----
        chunk_begin(0)
        w0l = []
        for k in range(K0):
            t_ = const.tile([128, D1], F16, tag=f"w0l{k}", name=f"w0l{k}")
            nc.sync.dma_start(t_[:], w0ld[k * 128:(k + 1) * 128, :])
            w0l.append(t_)
        for m in range(M0):
            chunk_m(0, m)
        if N_CH > 1:
            chunk_begin(1)
            for m in range(M0):
                chunk_m(1, m)
        a_dve(0)
        for t in range(T + 6):
            if 0 <= t + 1 < T:
                a_dve(t + 1)
            if 0 <= t - 6 < T:
                c3(t - 6)
            if 0 <= t < T:
                a_mm(t)
            if 0 <= t - 2 < T:
                b1(t - 2)
            if 0 <= t - 3 < T:
                b2(t - 3)
            if 0 <= t - 4 < T:
                c1(t - 4)
            if 0 <= t - 5 < T:
                c2(t - 5)
            c = t // CH + 2
            i = t % CH
            if c < N_CH:
                if i == 0:
                    chunk_begin(c)
                if i % 2 == 0:
                    chunk_m(c, i // 2)

        # un-shift layer-3 membrane: m3 = u3 + 20*b3, then DMA out
        nc.vector.tensor_tensor(mem_acc[:, D4:(T + 1) * D4],
                                mem_acc[:, D4:(T + 1) * D4],
                                k3_sb[:], ALU.add)
        nc.sync.dma_start(ospk[:], spk_acc[:, D4:(T + 1) * D4])
        nc.sync.dma_start(omem[:], mem_acc[:, D4:(T + 1) * D4])

    nc.compile()
    return nc


def _get_nc():
    if 'nc' not in _CACHE:
        _CACHE['nc'] = _build()
    return _CACHE['nc']


def _pair16(W):
    # W fp32 [K, N] -> (hi fp16, lo*2^11 fp16)
    h = W.astype(np.float16)
    l = ((W - h.astype(np.float32)) * np.float32(2048.0)).astype(np.float16)
    return h, l


def _prep(inputs):
    x = np.asarray(inputs["x"], dtype=np.float32)
    W = {k: np.asarray(inputs[k], dtype=np.float32)
         for k in ["W0", "b0", "W1", "b1", "W2", "b2", "W3", "b3"]}

    def fold(v, nf):
        # [nf*128] -> [128, nf*32]; col j*32+b holds v[j*128+p]
        return np.ascontiguousarray(
            np.repeat(v.reshape(nf, 128).T, BC, axis=1)).astype(np.float32)

    w0h, w0l = _pair16(np.ascontiguousarray(W["W0"].T))
    w1h, w1l = _pair16(np.ascontiguousarray(W["W1"].T))
    w2h, w2l = _pair16(np.ascontiguousarray(W["W2"].T))
    w3h, w3l = _pair16(np.ascontiguousarray(W["W3"].T))
    b1, b2, b3 = W["b1"], W["b2"], W["b3"]
    twenty = np.float32(20.0)
    common = dict(
        w0h=w0h, w0l=w0l, w1h=w1h, w1l=w1l, w2h=w2h, w2l=w2l,
        w3h=w3h, w3l=w3l,
        b0r=np.ascontiguousarray(W["b0"].reshape(M0, 128).T),
        th1f=fold(np.float32(1.0) - twenty * b1, F1),
        a1f=fold(np.float32(-19.0) * b1, F1),
        th2f=fold(np.float32(1.0) - twenty * b2, F2),
        a2f=fold(np.float32(-19.0) * b2, F2),
        th3r=np.ascontiguousarray(
            np.broadcast_to(np.float32(1.0) - twenty * b3, (BC, D4))),
        a3i=np.ascontiguousarray(
            np.broadcast_to(-twenty * b3, (BC, D4))),
        k3r=np.ascontiguousarray(
            np.broadcast_to(twenty * b3, (BC, T, D4)).reshape(BC, T * D4)),
        eye=np.eye(BC, dtype=np.float32),
    )
    in_maps = []
    for c in range(N_CORES):
        xc = x[c * BC:(c + 1) * BC]                       # [32, 100, 1024]
        xT = np.ascontiguousarray(
            xc.transpose(2, 1, 0).reshape(D0, T * BC))    # [1024, 3200]
        xh = xT.astype(np.float16)
        xl = (xT - xh.astype(np.float32)).astype(np.float16)
        xhs = (xh.astype(np.float32) * np.float32(2.0 ** -11)).astype(np.float16)
        in_maps.append(dict(common, x1=xh, x2=xl, x1s=xhs))
    return in_maps


def run(inputs, trace=False):
    in_maps = _prep(inputs)
    br = run_bass_kernel_spmd(_get_nc(), in_maps, list(range(N_CORES)),
                              trace=trace)
    spk = np.concatenate(
        [r["ospk"].reshape(BC, T, D4) for r in br.results], axis=0)
    mem = np.concatenate(
        [r["omem"].reshape(BC, T, D4) for r in br.results], axis=0)
    return (spk, mem), br


def kernel(**inputs):
    out, _ = run(inputs, trace=False)
    return out


# revision 13
# speedup vs baseline: 1.0183x; 1.0183x over previous
"""Trainium2 Bass kernel for a 4-layer LIF spiking net (snnTorch-style Leaky).

Reference (per step t, B=256, T=100):
    cur0 = x_t @ W0.T + b0 ; lif0 (1024)
    cur1 = s0 @ W1.T + b1  ; lif1 (512)
    cur2 = s1 @ W2.T + b2  ; lif2 (256)
    cur3 = s2 @ W3.T + b3  ; lif3 (10)   -> outputs (s3, m3) per step
    lif:  m' = 0.95*m + cur - H(m_prev - 1);  s = H(m' - 1)

Strategy: data-parallel over batch (32/core x 8 cores).
 - Numerics: all matmuls run on the PE in fp16 *pair* form, which is
   accurate to ~fp32 rounding class: operands are split hi/lo
   (a = a_h + a_l with a_h = fp16(a)), weights as W = W_h + W_l*2^-11
   with W_l stored prescaled by 2^11 (keeps it fp16-normal).  Spikes are
   {0,1} (exact in fp16), so layers 1-3 need only 2 passes; layer 0
   needs 3 (x_h*W_h, x_l*W_h, (x_h*2^-11)*(W_l*2^11)).
 - Phase 1 (time-parallel): CUR0 = W0 @ x.T + b0 for all t, produced in
   16-step chunks into an SBUF ring, interleaved with phase 2 steps.
 - Phase 2 (sequential over t): feature-major activations [D, 32].
   Matmuls use spike tiles as the stationary operand (M=32) streaming
   W.T (N=512/256); batch-major outputs are transposed back to
   feature-major on the PE.  LIF state uses the bias-shift u = m - 20*b
   (per-feature thresholds 1-20*b), so no per-step bias work.
 - Layer 3 keeps state u3 = m3 - 20*b3 in the output accumulator; one
   final elementwise add (+20*b3) converts the whole mem history before
   DMA-out.
"""
import sys
import numpy as np
from contextlib import ExitStack

sys.path.insert(0, '/opt/trn_rl_repo')

from concourse import bass, bacc, mybir, tile  # noqa: E402
from concourse.bass_utils import run_bass_kernel_spmd  # noqa: E402

F32 = mybir.dt.float32
F16 = mybir.dt.float16
ALU = mybir.AluOpType
ACTF = mybir.ActivationFunctionType

N_CORES = 8
BC = 32                      # batch per core
T = 100
D0, D1, D2, D3, D4 = 1024, 1024, 512, 256, 10
BETA = 0.95
SC = float(2.0 ** -11)       # lo-pass spike scale
CH = 16                      # time steps per phase-1 chunk
N_CH = (T + CH - 1) // CH    # 7 chunks (6x16 + 1x4)

M0 = D1 // 128   # 8  phase-1 M tiles
K0 = D0 // 128   # 8  phase-1 K tiles
F1 = D2 // 128   # 4  layer-1 feature folds
F2 = D3 // 128   # 2  layer-2 feature folds

_CACHE = {}


def _build():
    nc = bacc.Bacc("TRN2", target_bir_lowering=False)

    def dram(name, shape, dt, out=False):
        return nc.declare_dram_parameter(name, shape, dt, isOutput=out)

    x1d = dram("x1", [D0, T * BC], F16)
    x2d = dram("x2", [D0, T * BC], F16)
    x1sd = dram("x1s", [D0, T * BC], F16)
    w0hd = dram("w0h", [D0, D1], F16)
    w0ld = dram("w0l", [D0, D1], F16)
    w1hd = dram("w1h", [D1, D2], F16)
    w1ld = dram("w1l", [D1, D2], F16)
    w2hd = dram("w2h", [D2, D3], F16)
    w2ld = dram("w2l", [D2, D3], F16)
    w3hd = dram("w3h", [D3, D4], F16)
    w3ld = dram("w3l", [D3, D4], F16)
    b0rd = dram("b0r", [128, M0], F32)
    th1d = dram("th1f", [128, F1 * BC], F32)
    a1d = dram("a1f", [128, F1 * BC], F32)
    th2d = dram("th2f", [128, F2 * BC], F32)
    a2d = dram("a2f", [128, F2 * BC], F32)
    th3d = dram("th3r", [BC, D4], F32)
    a3d = dram("a3i", [BC, D4], F32)
    k3d = dram("k3r", [BC, T * D4], F32)
    eyed = dram("eye", [BC, BC], F32)
    ospk = dram("ospk", [BC, T * D4], F32, out=True)
    omem = dram("omem", [BC, T * D4], F32, out=True)

    with tile.TileContext(nc) as tc, ExitStack() as ctx:
        const = ctx.enter_context(tc.tile_pool(name="const", bufs=1))
        state = ctx.enter_context(tc.tile_pool(name="state", bufs=1))
        ring = ctx.enter_context(tc.tile_pool(name="ring", bufs=3))
        xin = ctx.enter_context(tc.tile_pool(name="xin", bufs=8))
        sbt = ctx.enter_context(tc.tile_pool(name="sbt", bufs=5))
        ps = ctx.enter_context(tc.tile_pool(name="ps", bufs=1, space="PSUM"))

        # ---------------- constants ----------------
        def loadrows(name, src, n_k, width, dt):
            # one folded tile [128, n_k, width]; single DMA; index [:, k, :]
            t_ = const.tile([128, n_k, width], dt, tag=name, name=name)
            nc.sync.dma_start(
                t_[:], src.rearrange("(k p) c -> p k c", p=128))
            return [t_[:, k, :] for k in range(n_k)]

        w0h = loadrows("w0h", w0hd, K0, D1, F16)

        def load1(name, src, shape, dt):
            t_ = const.tile(shape, dt, tag=name, name=name)
            nc.sync.dma_start(t_[:], src[:])
            return t_

        b0r_sb = load1("b0r", b0rd, [128, M0], F32)
        th1_sb = load1("th1", th1d, [128, F1 * BC], F32)
        th2_sb = load1("th2", th2d, [128, F2 * BC], F32)
        th3_sb = load1("th3", th3d, [BC, D4], F32)
        k3_sb = load1("k3", k3d, [BC, T * D4], F32)
        eye_sb = load1("eye", eyed, [BC, BC], F32)

        # ---------------- state ----------------
        def ppair(name, shape, dt=F32):
            return [state.tile(shape, dt, tag=f"{name}{i}", name=f"{name}{i}")
                    for i in range(2)]

        u0 = ppair("u0", [128, M0 * BC])
        A0 = ppair("A0", [128, M0 * BC])
        s0h = ppair("s0h", [128, M0 * BC], F16)
        s0s = ppair("s0s", [128, M0 * BC], F16)
        u1 = ppair("u1", [128, F1 * BC])
        A1 = ppair("A1", [128, F1 * BC])
        s1h = ppair("s1h", [128, F1 * BC], F16)
        s1s = ppair("s1s", [128, F1 * BC], F16)
        u2 = ppair("u2", [128, F2 * BC])
        A2 = ppair("A2", [128, F2 * BC])
        s2h = ppair("s2h", [128, F2 * BC], F16)
        s2s = ppair("s2s", [128, F2 * BC], F16)
        spk_acc = state.tile([BC, (T + 1) * D4], F32, tag="spk_acc")
        mem_acc = state.tile([BC, (T + 1) * D4], F32, tag="mem_acc")

        nc.vector.memset(A0[1][:], 0.0)
        nc.sync.dma_start(A1[1][:], a1d[:])
        nc.sync.dma_start(A2[1][:], a2d[:])
        nc.vector.memset(spk_acc[:, 0:D4], 0.0)
        nc.sync.dma_start(mem_acc[:, 0:D4], a3d[:])

        ring_tiles = {}

        # deferred constants (not needed until step 0)
        w1h = loadrows("w1h", w1hd, M0, D2, F16)
        w1l = loadrows("w1l", w1ld, M0, D2, F16)
        w2h = loadrows("w2h", w2hd, F1, D3, F16)
        w2l = loadrows("w2l", w2ld, F1, D3, F16)
        w3h = loadrows("w3h", w3hd, F2, D4, F16)
        w3l = loadrows("w3l", w3ld, F2, D4, F16)

        # ---------------- phase-1 (emitted as per-m units) ----------------
        chunk_x = {}

        def chunk_begin(c):
            t0 = c * CH
            nT = min(CH, T - t0)
            n = nT * BC
            xt = []
            for src_ in (x1d, x2d, x1sd):
                t_ = xin.tile([128, K0, CH * BC], F16, tag="x", name="xt")
                nc.sync.dma_start(
                    t_[:, :, 0:n],
                    src_.rearrange("(k p) c -> p k c",
                                   p=128)[:, :, t0 * BC:t0 * BC + n])
                xt.append([t_[:, k, :] for k in range(K0)])
            chunk_x[c] = (xt, nT, n)
            cur = ring.tile([128, CH, M0, BC], F32, tag="cur0", name="cur")
            ring_tiles[c] = cur

        def chunk_m(c, m):
            xt, nT, n = chunk_x[c]
            cur = ring_tiles[c]
            pp = ps.tile([128, 512], F32, tag="ph1", bufs=2, name="pp")
            for k in range(K0):
                ms = slice(m * 128, (m + 1) * 128)
                nc.tensor.matmul(pp[:, 0:n], w0h[k][:, ms], xt[0][k][:, 0:n],
                                 start=(k == 0), stop=False)
                nc.tensor.matmul(pp[:, 0:n], w0h[k][:, ms], xt[1][k][:, 0:n],
                                 start=False, stop=False)
            for k in range(K0):
                ms = slice(m * 128, (m + 1) * 128)
                nc.tensor.matmul(pp[:, 0:n], w0l[k][:, ms], xt[2][k][:, 0:n],
                                 start=False, stop=(k == K0 - 1))
            nc.scalar.activation(
                cur[:, 0:nT, m, :],
                pp[:, 0:n].rearrange("p (a b) -> p a b", b=BC),
                ACTF.Identity, bias=b0r_sb[:, m:m + 1], scale=1.0)

        # ---------------- step stages ----------------
        pend1 = {}
        pend2 = {}
        pt1s = {}
        pt2s = {}
        pm3s = {}

        def a_dve(t):
            # layer-0 LIF (critical recurrence, runs 1 step ahead)
            cur = ring_tiles[t // CH]
            ti = t % CH
            w_, r_ = t % 2, (t + 1) % 2
            c0 = cur[:, ti, :, :].rearrange("p a b -> p (a b)")
            nc.vector.tensor_tensor(u0[w_][:], A0[r_][:], c0, ALU.add)
            nc.vector.tensor_scalar(s0h[w_][:], u0[w_][:], 1.0, None, ALU.is_gt)
            nc.scalar.activation(s0s[w_][:], s0h[w_][:], ACTF.Copy,
                                 bias=0.0, scale=SC)
            nc.vector.scalar_tensor_tensor(A0[w_][:], u0[w_][:], BETA, s0h[w_][:],
                                           ALU.mult, ALU.subtract)

        def a_mm(t):
            # layer-1 matmuls + PSUM->SBUF copy
            w_ = t % 2
            pc1 = ps.tile([32, 512], F32, tag="curX", bufs=2, name="pc1")
            for m in range(M0):
                bs = slice(m * BC, (m + 1) * BC)
                nc.tensor.matmul(pc1[:], s0h[w_][:, bs], w1h[m][:],
                                 start=(m == 0), stop=False)
                nc.tensor.matmul(pc1[:], s0s[w_][:, bs], w1l[m][:],
                                 start=False, stop=(m == M0 - 1))
            c1sb = sbt.tile([32, 512], F32, tag="c1sb", bufs=4, name="c1sb")
            nc.scalar.copy(c1sb[:], pc1[:])
            pend1[t] = c1sb

        def b1(t):
            # transpose cur1 to feature-major
            c1sb = pend1.pop(t)
            pt1 = ps.tile([128, 128], F32, tag="fm", bufs=4, name="pt1")
            for j in range(F1):
                nc.tensor.transpose(pt1[:, j * BC:(j + 1) * BC],
                                    c1sb[:, j * 128:(j + 1) * 128],
                                    eye_sb[:])
            pt1s[t] = pt1

        def b2(t):
            # layer-1 LIF + layer-2 matmuls + copy
            w_, r_ = t % 2, (t + 1) % 2
            pt1 = pt1s.pop(t)
            nc.vector.tensor_tensor(u1[w_][:], A1[r_][:], pt1[:], ALU.add)
            nc.vector.tensor_tensor(s1h[w_][:], u1[w_][:], th1_sb[:], ALU.is_gt)
            nc.scalar.activation(s1s[w_][:], s1h[w_][:], ACTF.Copy,
                                 bias=0.0, scale=SC)
            nc.vector.scalar_tensor_tensor(A1[w_][:], u1[w_][:], BETA, s1h[w_][:],
                                           ALU.mult, ALU.subtract)
            pc2 = ps.tile([32, 512], F32, tag="curX", bufs=2, name="pc2")
            for j in range(F1):
                bs = slice(j * BC, (j + 1) * BC)
                nc.tensor.matmul(pc2[:, 0:D3], s1h[w_][:, bs], w2h[j][:],
                                 start=(j == 0), stop=False)
                nc.tensor.matmul(pc2[:, 0:D3], s1s[w_][:, bs], w2l[j][:],
                                 start=False, stop=(j == F1 - 1))
            c2sb = sbt.tile([32, D3], F32, tag="c2sb", bufs=4, name="c2sb")
            nc.scalar.copy(c2sb[:], pc2[:, 0:D3])
            pend2[t] = c2sb

        def c1(t):
            # transpose cur2
            c2sb = pend2.pop(t)
            pt2 = ps.tile([128, 128], F32, tag="fm", bufs=4, name="pt2")
            for j in range(F2):
                nc.tensor.transpose(pt2[:, j * BC:(j + 1) * BC],
                                    c2sb[:, j * 128:(j + 1) * 128],
                                    eye_sb[:])
            pt2s[t] = pt2

        def c2(t):
            # layer-2 LIF
            w_, r_ = t % 2, (t + 1) % 2
            pt2 = pt2s.pop(t)
            nc.vector.tensor_tensor(u2[w_][:], A2[r_][:], pt2[:, 0:F2 * BC],
                                    ALU.add)
            nc.vector.tensor_tensor(s2h[w_][:], u2[w_][:], th2_sb[:], ALU.is_gt)
            nc.scalar.activation(s2s[w_][:], s2h[w_][:], ACTF.Copy,
                                 bias=0.0, scale=SC)
            nc.vector.scalar_tensor_tensor(A2[w_][:], u2[w_][:], BETA, s2h[w_][:],
                                           ALU.mult, ALU.subtract)

        def c3(t):
            # layer-3 matmuls + LIF (outputs)
            w_ = t % 2
            pm3 = ps.tile([128, 128], F32, tag="fm", bufs=4, name="pm3")
            for j in range(F2):
                bs = slice(j * BC, (j + 1) * BC)
                nc.tensor.matmul(pm3[0:BC, 0:D4], s2h[w_][:, bs], w3h[j][:],
                                 start=(j == 0), stop=False)
                nc.tensor.matmul(pm3[0:BC, 0:D4], s2s[w_][:, bs], w3l[j][:],
                                 start=False, stop=(j == F2 - 1))
            prev = slice(t * D4, (t + 1) * D4)
            cursl = slice((t + 1) * D4, (t + 2) * D4)
            pre3 = sbt.tile([BC, D4], F32, tag="pre3", bufs=3, name="pre3")
            nc.vector.scalar_tensor_tensor(pre3[:], mem_acc[:, prev], BETA,
                                           spk_acc[:, prev],
                                           ALU.mult, ALU.subtract)
            nc.vector.tensor_tensor(mem_acc[:, cursl], pre3[:],
                                    pm3[0:BC, 0:D4], ALU.add)
            nc.vector.tensor_tensor(spk_acc[:, cursl], mem_acc[:, cursl],
                                    th3_sb[:], ALU.is_gt)

        # ---------------- schedule (software-pipelined, staged leads) ----
        chunk_begin(0)
        w0l = []
        for k in range(K0):
            t_ = const.tile([128, D1], F16, tag=f"w0l{k}", name=f"w0l{k}")
            nc.sync.dma_start(t_[:], w0ld[k * 128:(k + 1) * 128, :])
            w0l.append(t_)
        for m in range(M0):
            chunk_m(0, m)
        if N_CH > 1:
            chunk_begin(1)
            for m in range(M0):
                chunk_m(1, m)
        a_dve(0)
        for t in range(T + 6):
            if 0 <= t + 1 < T:
                a_dve(t + 1)
            if 0 <= t - 6 < T:
                c3(t - 6)
            if 0 <= t < T:
                a_mm(t)
            if 0 <= t - 2 < T:
                b1(t - 2)
            if 0 <= t - 3 < T:
                b2(t - 3)
            if 0 <= t - 4 < T:
                c1(t - 4)
            if 0 <= t - 5 < T:
                c2(t - 5)
            c = t // CH + 2
            i = t % CH
            if c < N_CH:
                if i == 0:
                    chunk_begin(c)
                if i % 2 == 0:
                    chunk_m(c, i // 2)

        # un-shift layer-3 membrane: m3 = u3 + 20*b3, then DMA out
        nc.vector.tensor_tensor(mem_acc[:, D4:(T + 1) * D4],
                                mem_acc[:, D4:(T + 1) * D4],
                                k3_sb[:], ALU.add)
        nc.sync.dma_start(ospk[:], spk_acc[:, D4:(T + 1) * D4])
        nc.sync.dma_start(omem[:], mem_acc[:, D4:(T + 1) * D4])

    nc.compile()
    return nc


def _get_nc():
    if 'nc' not in _CACHE:
        _CACHE['nc'] = _build()
    return _CACHE['nc']


def _pair16(W):
    # W fp32 [K, N] -> (hi fp16, lo*2^11 fp16)
    h = W.astype(np.float16)
    l = ((W - h.astype(np.float32)) * np.float32(2048.0)).astype(np.float16)
    return h, l


def _prep(inputs):
    x = np.asarray(inputs["x"], dtype=np.float32)
    W = {k: np.asarray(inputs[k], dtype=np.float32)
         for k in ["W0", "b0", "W1", "b1", "W2", "b2", "W3", "b3"]}

    def fold(v, nf):
        # [nf*128] -> [128, nf*32]; col j*32+b holds v[j*128+p]
        return np.ascontiguousarray(
            np.repeat(v.reshape(nf, 128).T, BC, axis=1)).astype(np.float32)

    w0h, w0l = _pair16(np.ascontiguousarray(W["W0"].T))
    w1h, w1l = _pair16(np.ascontiguousarray(W["W1"].T))
    w2h, w2l = _pair16(np.ascontiguousarray(W["W2"].T))
    w3h, w3l = _pair16(np.ascontiguousarray(W["W3"].T))
    b1, b2, b3 = W["b1"], W["b2"], W["b3"]
    twenty = np.float32(20.0)
    common = dict(
        w0h=w0h, w0l=w0l, w1h=w1h, w1l=w1l, w2h=w2h, w2l=w2l,
        w3h=w3h, w3l=w3l,
        b0r=np.ascontiguousarray(W["b0"].reshape(M0, 128).T),
        th1f=fold(np.float32(1.0) - twenty * b1, F1),
        a1f=fold(np.float32(-19.0) * b1, F1),
        th2f=fold(np.float32(1.0) - twenty * b2, F2),
        a2f=fold(np.float32(-19.0) * b2, F2),
        th3r=np.ascontiguousarray(
            np.broadcast_to(np.float32(1.0) - twenty * b3, (BC, D4))),
        a3i=np.ascontiguousarray(
            np.broadcast_to(-twenty * b3, (BC, D4))),
        k3r=np.ascontiguousarray(
            np.broadcast_to(twenty * b3, (BC, T, D4)).reshape(BC, T * D4)),
        eye=np.eye(BC, dtype=np.float32),
    )
    in_maps = []
    for c in range(N_CORES):
        xc = x[c * BC:(c + 1) * BC]                       # [32, 100, 1024]
        xT = np.ascontiguousarray(
            xc.transpose(2, 1, 0).reshape(D0, T * BC))    # [1024, 3200]
        xh = xT.astype(np.float16)
        xl = (xT - xh.astype(np.float32)).astype(np.float16)
        xhs = (xh.astype(np.float32) * np.float32(2.0 ** -11)).astype(np.float16)
        in_maps.append(dict(common, x1=xh, x2=xl, x1s=xhs))
    return in_maps


def run(inputs, trace=False):
    in_maps = _prep(inputs)
    br = run_bass_kernel_spmd(_get_nc(), in_maps, list(range(N_CORES)),
                              trace=trace)
    spk = np.concatenate(
        [r["ospk"].reshape(BC, T, D4) for r in br.results], axis=0)
    mem = np.concatenate(
        [r["omem"].reshape(BC, T, D4) for r in br.results], axis=0)
    return (spk, mem), br


def kernel(**inputs):
    out, _ = run(inputs, trace=False)
    return out
